# revision 7
# baseline (speedup 1.0000x reference)
"""Trainium2 Bass kernel for nn_Dense_369367187664.

Reference computation (B=8, S=2048, H=1024):
    x = hidden_states + pos_enc
    q = x @ Wq.T + bq
    k = sqi.T * Wk[:,0] + bk            (rank-1 "projection")
    v = x @ Wv.T + bv
    attn = softmax(q @ k.T / sqrt(H))
    context = mean(attn @ v, axis=seq)
    return context, attn

Because k is rank-1 in the feature dim, scores collapse algebraically:
    scores[i,j] = (a_i * sqi_j + c_i) / sqrt(H),   a = x @ (Wq.T @ w) + bq.w
with w = Wk[:,0]. The c_i term is constant along j and cancels in softmax, so
    attn[i,j] = softmax_j(a_i * sqi_j / sqrt(H))
Each attn row sums to 1, so the mean-pooled context collapses too:
    context = (1/S) * ((colsum(attn) @ x) @ Wv.T) + bv
This removes both S*S*H matmuls; the kernel is dominated by the S*S softmax
and the HBM write of attn itself.

Sharding: data-parallel over batch, one batch element per NeuronCore (B=8).
"""

import numpy as np

import concourse.bass as bass
import concourse.mybir as mybir
import concourse.tile as tile
from concourse import bass_utils

P = 128
S = 2048
H = 1024
NT = S // P   # 16 seq tiles
HO = H // P   # 8 feature chunks
FP32 = mybir.dt.float32
AF = mybir.ActivationFunctionType
ALU = mybir.AluOpType
RSQRT_H = 1.0 / 32.0  # 1/sqrt(1024)


def _split_sync_waits(nc):
    """Legalize sync waits for this container's walrus build, which accepts
    at most one sync-wait command per instruction (two for EventSemaphore).
    Extra waits move onto same-engine NOPs inserted just before the owner.
    """
    ctr = 0
    for fn in nc.m.functions:
        for blk in fn.blocks:
            out = []
            changed = False
            for ins in blk.instructions:
                si = ins.sync_info
                cap = 2 if isinstance(ins, mybir.InstEventSemaphore) else 1
                if si is not None and len(si.on_wait) > cap:
                    waits = list(si.on_wait)
                    for w in waits[cap:]:
                        ctr += 1
                        nop = mybir.InstNoOp(name=f"I-waitsplit-{ctr}")
                        nop.engine = ins.engine
                        nop.sync_info = mybir.SyncInfo(on_wait=[w], on_update=[])
                        out.append(nop)
                    ins.sync_info = mybir.SyncInfo(
                        on_wait=waits[:cap], on_update=list(si.on_update)
                    )
                    changed = True
                out.append(ins)
            if changed:
                blk.instructions = out


def _build():
    nc = bass.Bass(trn_type="TRN2", debug=False, num_devices=8)

    hs = nc.dram_tensor("hs", [S, H], FP32, kind="ExternalInput")
    sqi = nc.dram_tensor("sqi", [S], FP32, kind="ExternalInput")
    wq = nc.dram_tensor("wq", [H, H], FP32, kind="ExternalInput")
    wv = nc.dram_tensor("wv", [H, H], FP32, kind="ExternalInput")
    wk = nc.dram_tensor("wk", [H], FP32, kind="ExternalInput")
    bq = nc.dram_tensor("bq", [H], FP32, kind="ExternalInput")
    bv = nc.dram_tensor("bv", [H], FP32, kind="ExternalInput")
    attn = nc.dram_tensor("attn", [S, S], FP32, kind="ExternalOutput")
    ctx = nc.dram_tensor("ctx", [H], FP32, kind="ExternalOutput")

    hs_r = hs.ap().rearrange("(t p) h -> p t h", p=P)
    attn_r = attn.ap().rearrange("(t p) j -> p t j", p=P)

    with tile.TileContext(nc) as tc:
        with (
            tc.tile_pool(name="persist", bufs=1) as persist,
            tc.tile_pool(name="wqp", bufs=2) as wqp,
            tc.tile_pool(name="scratch", bufs=2) as scratch,
            tc.tile_pool(name="etile", bufs=3) as etile,
            tc.tile_pool(name="psum_acc", bufs=1, space="PSUM") as psum_acc,
            tc.tile_pool(name="psum_misc", bufs=1, space="PSUM") as psum_misc,
            tc.tile_pool(name="dram", bufs=1, space="DRAM") as dram,
        ):
            # ---- persistent tiles
            x_all = persist.tile([P, NT, H], FP32)        # 64 KiB/part
            wv_all = persist.tile([P, HO, H], FP32)       # 32 KiB/part
            t_bcast = persist.tile([P, S], FP32)          # 8 KiB/part
            u_bcast = persist.tile([P, H], FP32)          # 4 KiB/part
            w_rs = persist.tile([P, HO], FP32)
            bq_rs = persist.tile([P, HO], FP32)
            bv_rs = persist.tile([P, HO], FP32)
            u32_sb = persist.tile([1, H], FP32)
            s32_sb = persist.tile([1, 1], FP32)
            s32b = persist.tile([P, 1], FP32)
            a0_all = persist.tile([P, NT], FP32)
            a_all = persist.tile([P, NT], FP32)
            z_all = persist.tile([P, NT], FP32)
            rz_all = persist.tile([P, NT], FP32)
            cs_sb = persist.tile([1, S], FP32)
            cs_rs = persist.tile([P, NT], FP32)
            y_sb = persist.tile([1, H], FP32)
            y_bcast = persist.tile([P, H], FP32)
            ones_col = persist.tile([P, 1], FP32)
            ctx0_sb = persist.tile([P, HO], FP32)
            ctx_sb = persist.tile([P, HO], FP32)

            nc.vector.memset(ones_col[:], 1.0)

            # ---- prologue loads
            nc.sync.dma_start(w_rs[:], wk.ap().rearrange("(c p) -> p c", p=P))
            nc.sync.dma_start(bq_rs[:], bq.ap().rearrange("(c p) -> p c", p=P))
            nc.sync.dma_start(bv_rs[:], bv.ap().rearrange("(c p) -> p c", p=P))
            # sqi broadcast straight from DRAM to all 128 partitions
            nc.sync.dma_start(t_bcast[:], sqi.ap()[None, :].to_broadcast([P, S]))
            for it in range(NT):
                nc.sync.dma_start(x_all[:, it], hs_r[:, it])
            for c in range(HO):
                nc.sync.dma_start(wv_all[:, c], wv.ap()[c * P:(c + 1) * P, :])

            # ---- u = Wq.T @ w, s0 = bq . w   (contraction over rows of Wq)
            psum_u = psum_misc.tile([1, H], FP32, tag="psum_u_y")
            psum_s0 = psum_misc.tile([1, 1], FP32, tag="psum_s0")
            for ko in range(HO):
                wq_tile = wqp.tile([P, H], FP32)
                nc.sync.dma_start(wq_tile[:], wq.ap()[ko * P:(ko + 1) * P, :])
                for nh in range(2):
                    nc.tensor.matmul(
                        psum_u[:, nh * 512:(nh + 1) * 512],
                        w_rs[:, ko:ko + 1],
                        wq_tile[:, nh * 512:(nh + 1) * 512],
                        start=(ko == 0),
                        stop=(ko == HO - 1),
                    )
                nc.tensor.matmul(
                    psum_s0[:],
                    w_rs[:, ko:ko + 1],
                    bq_rs[:, ko:ko + 1],
                    start=(ko == 0),
                    stop=(ko == HO - 1),
                )
            # fold 1/sqrt(H) into u and s0
            nc.scalar.mul(u32_sb[:], psum_u[:], RSQRT_H)
            nc.scalar.mul(s32_sb[:], psum_s0[:], RSQRT_H)

            # ---- broadcasts (row -> all partitions) via DRAM round-trip
            u_dram = dram.tile([H], FP32)
            s_dram = dram.tile([1], FP32)
            nc.sync.dma_start(u_dram[:].rearrange("(o s) -> o s", o=1), u32_sb[:])
            nc.sync.dma_start(s_dram[:].rearrange("(o s) -> o s", o=1), s32_sb[:])
            nc.sync.dma_start(u_bcast[:], u_dram[:][None, :].to_broadcast([P, H]))
            nc.sync.dma_start(s32b[:], s_dram[:][None, :].to_broadcast([P, 1]))

            # ---- main loop over 16 row-tiles of attn
            psum_cs = psum_acc.tile([1, S], FP32)  # colsum accumulator
            for it in range(NT):
                # a_i = (x_i . u + bq.w) / sqrt(H)   (scale folded into u,s0)
                stt_out = scratch.tile([P, H], FP32, tag="ttr")
                nc.vector.scalar_tensor_tensor(
                    out=stt_out[:],
                    in0=x_all[:, it],
                    scalar=1.0,
                    in1=u_bcast[:],
                    op0=ALU.mult,
                    op1=ALU.mult,
                    accum_out=a0_all[:, it:it + 1],
                )
                nc.vector.tensor_scalar_add(
                    a_all[:, it:it + 1], a0_all[:, it:it + 1], s32b[:]
                )
                # e = exp(a_i * sqi_j), z_i = sum_j e   in one ACT pass
                e_tile = etile.tile([P, S], FP32, tag="e")
                nc.scalar.activation(
                    e_tile[:],
                    t_bcast[:],
                    AF.Exp,
                    scale=a_all[:, it:it + 1],
                    accum_out=z_all[:, it:it + 1],
                )
                nc.vector.reciprocal(rz_all[:, it:it + 1], z_all[:, it:it + 1])
                # normalize rows in place -> attn tile
                nc.vector.tensor_scalar_mul(
                    e_tile[:], e_tile[:], rz_all[:, it:it + 1]
                )
                # colsum += ones.T @ attn_tile (contract over partitions)
                for jc in range(4):
                    nc.tensor.matmul(
                        psum_cs[:, jc * 512:(jc + 1) * 512],
                        ones_col[:],
                        e_tile[:, jc * 512:(jc + 1) * 512],
                        start=(it == 0),
                        stop=(it == NT - 1),
                    )
                nc.sync.dma_start(attn_r[:, it], e_tile[:])

            # ---- colsum -> [P, NT] layout via DRAM round-trip
            nc.vector.tensor_copy(cs_sb[:], psum_cs[:])
            cs_dram = dram.tile([S], FP32)
            nc.sync.dma_start(cs_dram[:].rearrange("(o s) -> o s", o=1), cs_sb[:])
            nc.sync.dma_start(cs_rs[:], cs_dram[:].rearrange("(t p) -> p t", p=P))

            # ---- y = colsum @ x  (contract over seq)
            psum_y = psum_misc.tile([1, H], FP32, tag="psum_u_y")
            for it in range(NT):
                for nh in range(2):
                    nc.tensor.matmul(
                        psum_y[:, nh * 512:(nh + 1) * 512],
                        cs_rs[:, it:it + 1],
                        x_all[:, it, nh * 512:(nh + 1) * 512],
                        start=(it == 0),
                        stop=(it == NT - 1),
                    )
            nc.scalar.copy(y_sb[:], psum_y[:])
            y_dram = dram.tile([H], FP32)
            nc.sync.dma_start(y_dram[:].rearrange("(o s) -> o s", o=1), y_sb[:])
            nc.sync.dma_start(y_bcast[:], y_dram[:][None, :].to_broadcast([P, H]))

            # ---- context[o] = bv[o] + (1/S) * sum_h Wv[o,h] y[h]
            for c in range(HO):
                stt_out2 = scratch.tile([P, H], FP32, tag="ttr")
                nc.vector.scalar_tensor_tensor(
                    out=stt_out2[:],
                    in0=wv_all[:, c],
                    scalar=1.0 / S,
                    in1=y_bcast[:],
                    op0=ALU.mult,
                    op1=ALU.mult,
                    accum_out=ctx0_sb[:, c:c + 1],
                )
                nc.vector.tensor_scalar_add(
                    ctx_sb[:, c:c + 1], ctx0_sb[:, c:c + 1], bv_rs[:, c:c + 1]
                )
            nc.sync.dma_start(ctx.ap().rearrange("(c p) -> p c", p=P), ctx_sb[:])

    _split_sync_waits(nc)
    return nc


_NC_CACHE = None


def _get_nc():
    global _NC_CACHE
    if _NC_CACHE is None:
        _NC_CACHE = _build()
    return _NC_CACHE


def _make_in_maps(inputs):
    hidden_states = np.asarray(inputs["hidden_states"], dtype=np.float32)
    sqi_sequence = np.asarray(inputs["sqi_sequence"], dtype=np.float32)
    pos_enc = np.asarray(inputs["pos_enc"], dtype=np.float32)
    B = hidden_states.shape[0]
    assert hidden_states.shape == (B, S, H) and B == 8

    # pos_enc is an additive term on x; it is all-zeros for this problem's
    # input distribution. Fold it on the host only if it is ever nonzero.
    if np.any(pos_enc):
        x_host = hidden_states + pos_enc[:, :S, :]
    else:
        x_host = hidden_states

    wq_np = np.ascontiguousarray(np.asarray(inputs["Wq"], dtype=np.float32))
    wv_np = np.ascontiguousarray(np.asarray(inputs["Wv"], dtype=np.float32))
    wk_np = np.ascontiguousarray(np.asarray(inputs["Wk"], dtype=np.float32)[:, 0])
    bq_np = np.ascontiguousarray(np.asarray(inputs["bq"], dtype=np.float32))
    bv_np = np.ascontiguousarray(np.asarray(inputs["bv"], dtype=np.float32))
    # bk shifts every score in a row by the same amount -> cancels in softmax
    # and does not reach context; it is unused.

    in_maps = []
    for b in range(B):
        in_maps.append({
            "hs": np.ascontiguousarray(x_host[b]),
            "sqi": np.ascontiguousarray(sqi_sequence[b, 0]),
            "wq": wq_np,
            "wv": wv_np,
            "wk": wk_np,
            "bq": bq_np,
            "bv": bv_np,
        })
    return in_maps


def kernel(hidden_states, sqi_sequence, pos_enc, Wq, bq, Wk, bk, Wv, bv):
    in_maps = _make_in_maps(dict(
        hidden_states=hidden_states, sqi_sequence=sqi_sequence,
        pos_enc=pos_enc, Wq=Wq, bq=bq, Wk=Wk, bk=bk, Wv=Wv, bv=bv,
    ))
    B = len(in_maps)
    nc = _get_nc()
    res = bass_utils.run_bass_kernel_spmd(nc, in_maps, core_ids=list(range(B)))
    attn = np.stack([res.results[b]["attn"] for b in range(B)])
    context = np.stack([res.results[b]["ctx"] for b in range(B)])
    return context, attn


# revision 32
# speedup vs baseline: 38.7700x; 38.7700x over previous
"""Trainium2 Bass kernel for nn_Dense_369367187664.

Reference computation (B=8, S=2048, H=1024):
    x = hidden_states + pos_enc
    q = x @ Wq.T + bq
    k = sqi.T * Wk[:,0] + bk            (rank-1 "projection")
    v = x @ Wv.T + bv
    attn = softmax(q @ k.T / sqrt(H))
    context = mean(attn @ v, axis=seq)
    return context, attn

Because k is rank-1 in the feature dim, scores collapse algebraically:
    scores[i,j] = (a_i * sqi_j + c_i) / sqrt(H),   a = x @ (Wq.T @ w) + bq.w
with w = Wk[:,0]. The c_i term is constant along j and cancels in softmax, so
    attn[i,j] = softmax_j(a_i * sqi_j / sqrt(H))
Each attn row sums to 1, so the mean-pooled context collapses too:
    context = (1/S) * ((colsum(attn) @ x) @ Wv.T) + bv
This removes both S*S*H matmuls; the kernel is dominated by the S*S softmax
and the HBM write of attn itself.

Sharding: data-parallel over batch, one batch element per NeuronCore (B=8).
"""

import numpy as np

import concourse.bass as bass
import concourse.mybir as mybir
import concourse.tile as tile
from concourse import bass_utils

P = 128
S = 2048
H = 1024
NT = S // P   # 16 seq tiles
HO = H // P   # 8 feature chunks
FP32 = mybir.dt.float32
FP32R = mybir.dt.float32r
AF = mybir.ActivationFunctionType
ALU = mybir.AluOpType
RSQRT_H = 1.0 / 32.0  # 1/sqrt(1024)


def _split_sync_waits(nc):
    """Legalize sync waits for this container's walrus build, which accepts
    at most one sync-wait command per instruction (two for EventSemaphore).
    Extra waits move onto same-engine NOPs inserted just before the owner.
    """
    ctr = 0
    for fn in nc.m.functions:
        for blk in fn.blocks:
            out = []
            changed = False
            for ins in blk.instructions:
                si = ins.sync_info
                cap = 2 if isinstance(ins, mybir.InstEventSemaphore) else 1
                if si is not None and len(si.on_wait) > cap:
                    waits = list(si.on_wait)
                    for w in waits[cap:]:
                        ctr += 1
                        nop = mybir.InstNoOp(name=f"I-waitsplit-{ctr}")
                        nop.engine = ins.engine
                        nop.sync_info = mybir.SyncInfo(on_wait=[w], on_update=[])
                        out.append(nop)
                    ins.sync_info = mybir.SyncInfo(
                        on_wait=waits[:cap], on_update=list(si.on_update)
                    )
                    changed = True
                out.append(ins)
            if changed:
                blk.instructions = out


def _build(variant="full", repeats=1, chain=None):
    # variant flags for performance bisection; `repeats` replicates the whole
    # computation inside one NEFF so per-launch overhead can be differenced
    # out when timing. `chain` (default on when repeats > 1) threads a zero
    # contribution of each repeat's attn output into the next repeat's math,
    # so no repeat's stores can be dead-store-eliminated and repeats run
    # back-to-back without overlap — a faithful per-invocation time.
    if chain is None:
        chain = repeats > 1
    do_exp = variant not in ("no_exp",)
    do_norm = variant not in ("no_exp", "no_norm")
    do_colsum = variant not in ("no_exp", "no_norm", "no_colsum")
    do_out = variant not in ("no_out",)
    out_scratch = variant == "out_scratch"
    do_stt = variant not in ("no_stt",)
    do_ytail = variant not in ("no_ytail",)
    dma_only = variant == "dma_only"
    if dma_only:
        do_exp = do_norm = do_colsum = do_stt = False

    nc = bass.Bass(trn_type="TRN2", debug=False, num_devices=8)

    hs = nc.dram_tensor("hs", [S, H], FP32, kind="ExternalInput")
    sqi = nc.dram_tensor("sqi", [S], FP32, kind="ExternalInput")
    wq = nc.dram_tensor("wq", [H, H], FP32, kind="ExternalInput")
    wv = nc.dram_tensor("wv", [H, H], FP32, kind="ExternalInput")
    wk = nc.dram_tensor("wk", [H], FP32, kind="ExternalInput")
    bq = nc.dram_tensor("bq", [H], FP32, kind="ExternalInput")
    bv = nc.dram_tensor("bv", [H], FP32, kind="ExternalInput")
    attn = nc.dram_tensor("attn", [S, S], FP32, kind="ExternalOutput")
    ctx = nc.dram_tensor("ctx", [H], FP32, kind="ExternalOutput")

    hs_r = hs.ap().rearrange("(t p) h -> p t h", p=P)
    attn_r = attn.ap().rearrange("(t p) j -> p t j", p=P)

    with tile.TileContext(nc) as tc:
        with (
            tc.tile_pool(name="persist", bufs=1) as persist,
            tc.tile_pool(name="wqp", bufs=1) as wqp,
            tc.tile_pool(name="scratch", bufs=2) as scratch,
            tc.tile_pool(name="etile", bufs=3) as etile,
            tc.tile_pool(name="psum_acc", bufs=1, space="PSUM") as psum_acc,
            tc.tile_pool(name="psum_misc", bufs=1, space="PSUM") as psum_misc,
            tc.tile_pool(name="dram", bufs=1, space="DRAM") as dram,
        ):
          carry = None
          for _rep in range(repeats):
            # ---- persistent tiles
            x_all = persist.tile([P, NT, H], FP32)        # 64 KiB/part
            wv_all = persist.tile([P, HO, H], FP32)       # 32 KiB/part
            t_bcast = persist.tile([P, S], FP32)          # 8 KiB/part
            u_bcast = persist.tile([P, H], FP32)          # 4 KiB/part
            w_rs = persist.tile([P, HO], FP32)
            bq_rs = persist.tile([P, HO], FP32)
            bv_rs = persist.tile([P, HO], FP32)
            u32_sb = persist.tile([1, H], FP32)
            s32_sb = persist.tile([1, 1], FP32)
            s32b = persist.tile([P, 1], FP32)
            a0_all = persist.tile([P, NT], FP32)
            a_all = persist.tile([P, NT], FP32)
            z_all = persist.tile([P, NT], FP32)
            rz_all = persist.tile([P, NT], FP32)
            cs_sb = persist.tile([1, S], FP32)
            cs_rs = persist.tile([P, NT], FP32)
            y_sb = persist.tile([1, H], FP32)
            y_bcast = persist.tile([P, H], FP32)
            ones_col = persist.tile([P, 1], FP32)
            ones_row = persist.tile([1, P], FP32)
            t_sb = persist.tile([1, S], FP32)
            ctx0_sb = persist.tile([P, HO], FP32)
            ctx_sb = persist.tile([P, HO], FP32)

            nc.vector.memset(ones_col[:], 1.0)
            nc.vector.memset(ones_row[:], 1.0)

            # ---- prologue loads, in queue order: everything gating the
            # first exp goes first (w, bq, Wq, sqi), then x, then Wv (only
            # needed at the tail). Inputs ride the SP HWDGE queue; attn-out
            # stores ride the ACT queue so a compute-gated store can never
            # head-block an input load.
            nc.sync.dma_start(w_rs[:], wk.ap().rearrange("(c p) -> p c", p=P))
            nc.sync.dma_start(bq_rs[:], bq.ap().rearrange("(c p) -> p c", p=P))
            wq_all = wqp.tile([P, HO, H], FP32)
            wq_r = wq.ap().rearrange("(c p) h -> p c h", p=P)
            for kg in range(HO // 2):
                nc.sync.dma_start(
                    wq_all[:, kg * 2:(kg + 1) * 2], wq_r[:, kg * 2:(kg + 1) * 2]
                )
            nc.sync.dma_start(t_sb[:], sqi.ap().rearrange("(o s) -> o s", o=1))
            for it4 in range(NT // 4):
                nc.sync.dma_start(
                    x_all[:, it4 * 4:(it4 + 1) * 4], hs_r[:, it4 * 4:(it4 + 1) * 4]
                )
            nc.sync.dma_start(bv_rs[:], bv.ap().rearrange("(c p) -> p c", p=P))
            nc.sync.dma_start(
                wv_all[:], wv.ap().rearrange("(c p) h -> p c h", p=P)
            )

            # ---- t broadcast via PE (K=1 matmul with a ones row replicates
            # partition 0 across all 128 partitions). This doubles as the HAM
            # warm-up for the PE while the Wq DMA is still in flight.
            for tw in range(4):
                psum_bc = psum_misc.tile([P, 512], FP32, tag="psum_bc")
                nc.tensor.matmul(
                    psum_bc[:], ones_row[:], t_sb[:, tw * 512:(tw + 1) * 512],
                    start=True, stop=True,
                )
                nc.vector.tensor_copy(t_bcast[:, tw * 512:(tw + 1) * 512],
                                      psum_bc[:])

            # ---- u = Wq.T @ w, s0 = bq . w   (contraction over rows of Wq)
            psum_u = psum_misc.tile([1, H], FP32, tag="psum_u_y")
            psum_s0 = psum_misc.tile([1, 1], FP32, tag="psum_s0")
            for ko in range(HO):
                for nh in range(2):
                    nc.tensor.matmul(
                        psum_u[:, nh * 512:(nh + 1) * 512],
                        w_rs[:, ko:ko + 1],
                        wq_all[:, ko, nh * 512:(nh + 1) * 512],
                        start=(ko == 0),
                        stop=(ko == HO - 1),
                    )
                nc.tensor.matmul(
                    psum_s0[:],
                    w_rs[:, ko:ko + 1],
                    bq_rs[:, ko:ko + 1],
                    start=(ko == 0),
                    stop=(ko == HO - 1),
                )
            # fold 1/sqrt(H) into u and s0
            nc.scalar.mul(u32_sb[:], psum_u[:], RSQRT_H)
            nc.scalar.mul(s32_sb[:], psum_s0[:], RSQRT_H)
            if chain and carry is not None:
                # += 0 * (previous repeat's attn probe): forces the previous
                # repeat's stores live and serializes repeats.
                nc.vector.tensor_scalar_add(s32_sb[:], s32_sb[:], carry[:1, :1])

            # ---- broadcast u and s32 to all partitions via PE
            for uw in range(2):
                psum_bc = psum_misc.tile([P, 512], FP32, tag="psum_bc")
                nc.tensor.matmul(
                    psum_bc[:], ones_row[:], u32_sb[:, uw * 512:(uw + 1) * 512],
                    start=True, stop=True,
                )
                nc.vector.tensor_copy(u_bcast[:, uw * 512:(uw + 1) * 512],
                                      psum_bc[:])
            psum_bc1 = psum_misc.tile([P, 1], FP32, tag="psum_bc")
            nc.tensor.matmul(psum_bc1[:], ones_row[:], s32_sb[:],
                             start=True, stop=True)
            nc.vector.tensor_copy(s32b[:], psum_bc1[:])

            # ---- main loop over 16 row-tiles of attn
            attn_scr = dram.tile([S, S], FP32) if out_scratch else None
            psum_cs = psum_acc.tile([1, S], FP32)  # colsum accumulator
            for it in range(NT):
                # a_i = (x_i . u + bq.w) / sqrt(H)   (scale folded into u,s0)
                if do_stt:
                    stt_out = scratch.tile([P, H], FP32, tag="ttr")
                    nc.vector.scalar_tensor_tensor(
                        out=stt_out[:],
                        in0=x_all[:, it],
                        scalar=1.0,
                        in1=u_bcast[:],
                        op0=ALU.mult,
                        op1=ALU.mult,
                        accum_out=a0_all[:, it:it + 1],
                    )
                    nc.vector.tensor_scalar_add(
                        a_all[:, it:it + 1], a0_all[:, it:it + 1], s32b[:]
                    )
                e_tile = etile.tile([P, S], FP32, tag="e")
                if do_exp:
                    # e = exp(a_i * sqi_j), z_i = sum_j e   in one ACT pass
                    nc.scalar.activation(
                        e_tile[:],
                        t_bcast[:],
                        AF.Exp,
                        scale=a_all[:, it:it + 1],
                        accum_out=z_all[:, it:it + 1],
                    )
                else:
                    nc.vector.memset(e_tile[:, :1], 1.0)
                if do_norm:
                    nc.vector.reciprocal(rz_all[:, it:it + 1], z_all[:, it:it + 1])
                    # normalize rows in place -> attn tile
                    nc.vector.tensor_scalar_mul(
                        e_tile[:], e_tile[:], rz_all[:, it:it + 1]
                    )
                if do_colsum:
                    # colsum += ones.T @ attn_tile (contract over partitions).
                    # float32r runs the PE at full rate (fp32 is 1/4); it is
                    # reduced-precision but only feeds the context path.
                    for jc in range(4):
                        nc.tensor.matmul(
                            psum_cs[:, jc * 512:(jc + 1) * 512],
                            ones_col[:],
                            e_tile[:, jc * 512:(jc + 1) * 512],
                            start=(it == 0),
                            stop=(it == NT - 1),
                        )
                if do_out:
                    dst = (attn_scr[:].rearrange("(t p) j -> p t j", p=P)[:, it]
                           if out_scratch else attn_r[:, it])
                    nc.scalar.dma_start(dst, e_tile[:])

            if do_colsum and do_ytail:
                # ---- colsum -> [P, NT] layout via DRAM round-trip
                nc.vector.tensor_copy(cs_sb[:], psum_cs[:])
                cs_dram = dram.tile([S], FP32)
                nc.sync.dma_start(cs_dram[:].rearrange("(o s) -> o s", o=1), cs_sb[:])
                nc.sync.dma_start(cs_rs[:], cs_dram[:].rearrange("(t p) -> p t", p=P))

                # ---- y = colsum @ x  (contract over seq)
                psum_y = psum_misc.tile([1, H], FP32, tag="psum_u_y")
                for it in range(NT):
                    for nh in range(2):
                        nc.tensor.matmul(
                            psum_y[:, nh * 512:(nh + 1) * 512],
                            cs_rs[:, it:it + 1],
                            x_all[:, it, nh * 512:(nh + 1) * 512],
                            start=(it == 0),
                            stop=(it == NT - 1),
                        )
                nc.scalar.copy(y_sb[:], psum_y[:])
                for yw in range(2):
                    psum_bc2 = psum_misc.tile([P, 512], FP32, tag="psum_bc")
                    nc.tensor.matmul(
                        psum_bc2[:], ones_row[:],
                        y_sb[:, yw * 512:(yw + 1) * 512],
                        start=True, stop=True,
                    )
                    nc.vector.tensor_copy(y_bcast[:, yw * 512:(yw + 1) * 512],
                                          psum_bc2[:])

                # ---- context[o] = bv[o] + (1/S) * sum_h Wv[o,h] y[h]
                for c in range(HO):
                    stt_out2 = scratch.tile([P, H], FP32, tag="ttr")
                    nc.vector.scalar_tensor_tensor(
                        out=stt_out2[:],
                        in0=wv_all[:, c],
                        scalar=1.0 / S,
                        in1=y_bcast[:],
                        op0=ALU.mult,
                        op1=ALU.mult,
                        accum_out=ctx0_sb[:, c:c + 1],
                    )
                    nc.vector.tensor_scalar_add(
                        ctx_sb[:, c:c + 1], ctx0_sb[:, c:c + 1], bv_rs[:, c:c + 1]
                    )
            elif do_colsum:
                # no_ytail: consume colsum so it cannot be dead-code-eliminated
                nc.vector.tensor_copy(cs_sb[:], psum_cs[:])
                nc.sync.dma_start(
                    ctx.ap().rearrange("(o h) -> o h", o=1), cs_sb[:, :H]
                )
            else:
                nc.vector.memset(ctx_sb[:], 0.0)
            if not (do_colsum and not do_ytail):
                nc.sync.dma_start(ctx.ap().rearrange("(c p) -> p c", p=P), ctx_sb[:])

            if chain and do_out and not out_scratch and _rep < repeats - 1:
                rb = persist.tile([1, NT], FP32, tag="rb")
                carry = persist.tile([1, 1], FP32, tag="carry")
                nc.sync.dma_start(rb[:], attn_r[:1, :, 0])
                nc.vector.reduce_sum(carry[:], rb[:], axis=mybir.AxisListType.X)
                nc.vector.tensor_scalar_mul(carry[:], carry[:], 0.0)

    _split_sync_waits(nc)
    return nc


_NC_CACHE = None


def _get_nc():
    global _NC_CACHE
    if _NC_CACHE is None:
        _NC_CACHE = _build()
    return _NC_CACHE


def _make_in_maps(inputs):
    hidden_states = np.asarray(inputs["hidden_states"], dtype=np.float32)
    sqi_sequence = np.asarray(inputs["sqi_sequence"], dtype=np.float32)
    pos_enc = np.asarray(inputs["pos_enc"], dtype=np.float32)
    B = hidden_states.shape[0]
    assert hidden_states.shape == (B, S, H) and B == 8

    # pos_enc is an additive term on x; it is all-zeros for this problem's
    # input distribution. Fold it on the host only if it is ever nonzero.
    if np.any(pos_enc):
        x_host = hidden_states + pos_enc[:, :S, :]
    else:
        x_host = hidden_states

    wq_np = np.ascontiguousarray(np.asarray(inputs["Wq"], dtype=np.float32))
    wv_np = np.ascontiguousarray(np.asarray(inputs["Wv"], dtype=np.float32))
    wk_np = np.ascontiguousarray(np.asarray(inputs["Wk"], dtype=np.float32)[:, 0])
    bq_np = np.ascontiguousarray(np.asarray(inputs["bq"], dtype=np.float32))
    bv_np = np.ascontiguousarray(np.asarray(inputs["bv"], dtype=np.float32))
    # bk shifts every score in a row by the same amount -> cancels in softmax
    # and does not reach context; it is unused.

    in_maps = []
    for b in range(B):
        in_maps.append({
            "hs": np.ascontiguousarray(x_host[b]),
            "sqi": np.ascontiguousarray(sqi_sequence[b, 0]),
            "wq": wq_np,
            "wv": wv_np,
            "wk": wk_np,
            "bq": bq_np,
            "bv": bv_np,
        })
    return in_maps


def kernel(hidden_states, sqi_sequence, pos_enc, Wq, bq, Wk, bk, Wv, bv):
    in_maps = _make_in_maps(dict(
        hidden_states=hidden_states, sqi_sequence=sqi_sequence,
        pos_enc=pos_enc, Wq=Wq, bq=bq, Wk=Wk, bk=bk, Wv=Wv, bv=bv,
    ))
    B = len(in_maps)
    nc = _get_nc()
    res = bass_utils.run_bass_kernel_spmd(nc, in_maps, core_ids=list(range(B)))
    attn = np.stack([res.results[b]["attn"] for b in range(B)])
    context = np.stack([res.results[b]["ctx"] for b in range(B)])
    return context, attn
